# revision 53
# baseline (speedup 1.0000x reference)
"""Trainium2 Bass kernel for nn_Model_15590731285219 (GNN message passing).

Strategy (v2, bf16):
  - Dense masked attention instead of edge-list scatter (edge list == tsym>0).
  - 8 cores = 4 graphs x 2 row-halves; trunk replicated per pair, classifier
    row-split via a selection matmul (keeps the program SPMD-uniform).
  - bf16 weights + activations everywhere (PSUM/residual/LN stats in fp32):
    halves HBM traffic, 4x on 128-free matmuls, enables DVE 2x/4x modes.
  - Softmax without max-subtraction (scores are provably < 6 in magnitude);
    the -30000 edge mask is accumulated into the score PSUM by a PE
    identity-matmul; SCALE is folded into wq/bq on the host.
  - Q/K produced directly transposed (weight-stationary chunked matmuls), so
    no natural->transposed roundtrip; biases folded in via ones-row matmuls.
  - LayerNorm via bn_stats/bn_aggr + one ACT copy (scale=rstd, bias=-m*rstd).
  - Classifier: per-s column-scaled Cw (DVE/Pool tensor_scalar 4x mode) +
    half-width matmuls; u/v/eb1 terms folded into the same PSUM accumulation
    by ones/ident matmuls; sigmoid + symmetrization + masking on the host.
  - Per-layer weights double-buffered (bufs=2) so layer d+1 weight DMA
    overlaps layer d compute.
"""

from contextlib import ExitStack

import numpy as np
import ml_dtypes

import concourse.bass as bass
import concourse.tile as tile
import concourse.mybir as mybir
import concourse.bacc as bacc
from concourse.bass_utils import run_bass_kernel_spmd

B, N, H, NH, DEPTH = 4, 128, 512, 8, 4
HD = H // NH
MH = 4 * H
EHD = 64
SCALE = HD ** -0.5
NEGM = -30000.0
FC = H // 128
MC = MH // 128
NCORES = 8
ROWS = N // 2
NG = ROWS // 4          # classifier groups of 4 rows

f32 = mybir.dt.float32
bf16 = mybir.dt.bfloat16
i32 = mybir.dt.int32
AF = mybir.ActivationFunctionType
ALU = mybir.AluOpType
AX = mybir.AxisListType

BF = ml_dtypes.bfloat16

import os as _os
TRUNK_LAYERS = int(_os.environ.get("K_LAYERS", str(DEPTH)))
RUN_CLASSIFIER = _os.environ.get("K_CLS", "1") == "1"
K_DBG = _os.environ.get("K_DBG", "0") == "1"


def build_program():
    nc = bacc.Bacc("TRN2", target_bir_lowering=False, debug=False,
                   num_devices=NCORES)

    def din(name, shape, dt=bf16):
        return nc.dram_tensor(name, list(shape), dt, kind="ExternalInput")

    crow_d = din("crow", (1, 1984))
    cmat_d = din("cmat", (N, 1472))
    blockones_d = din("blockones", (4, 512))
    nemb_d = din("nemb", (N, H), f32)
    wqkvo_d = din("wqkvo", (DEPTH, 4, H, H))
    biases_d = din("biases", (DEPTH, 4608))
    w1_d = din("w1", (DEPTH, H, MH))
    w2_d = din("w2", (DEPTH, MH, H))
    ew1_d = din("ew1", (3 * H, EHD))
    ew2bd_d = din("ew2bd", (2 * EHD, 2 * EHD))
    ew3dd_d = din("ew3dd", (2 * EHD, 2))

    lout_d = nc.dram_tensor("lout", [16, 512], f32, kind="ExternalOutput")
    dbg = {}
    if K_DBG:
        for nm, shp in [("dbg_zv", (128, 512)), ("dbg_g1", (128, 512)),
                        ("dbg_g2", (128, 512)), ("dbg_u", (EHD, ROWS)),
                        ("dbg_v", (N, 2 * EHD)), ("dbg_xsel", (128, 4 * ROWS)),
                        ("dbg_xT", (128, 4 * 128))]:
            dbg[nm] = nc.dram_tensor(nm, list(shp), f32, kind="ExternalOutput")

    with tile.TileContext(nc) as tc, ExitStack() as ctx:
        pool_c = ctx.enter_context(tc.tile_pool(name="const", bufs=1))
        pool_w = ctx.enter_context(tc.tile_pool(name="wts", bufs=2))
        pool_a = ctx.enter_context(tc.tile_pool(name="acts", bufs=1))
        pool_t = ctx.enter_context(tc.tile_pool(name="temps", bufs=2))
        pool_x = ctx.enter_context(tc.tile_pool(name="xres", bufs=2))
        pool_sm = ctx.enter_context(tc.tile_pool(name="small", bufs=4))
        pool_o = ctx.enter_context(tc.tile_pool(name="outp", bufs=2))
        pool_pb = ctx.enter_context(tc.tile_pool(name="psb", bufs=3,
                                                 space="PSUM"))
        pool_ps = ctx.enter_context(tc.tile_pool(name="pss", bufs=5,
                                                 space="PSUM"))

        def ln_newton(var_eps):
            """rstd = 1/sqrt(var_eps): magic seed + 2 Newton steps, DVE only."""
            sh = pool_sm.tile([N, 1], i32, tag="ln_sh")
            nc.vector.tensor_scalar(sh[:], var_eps.bitcast(i32), 1, None,
                                    ALU.arith_shift_right)
            y0i = pool_sm.tile([N, 1], i32, tag="ln_y0i")
            nc.vector.tensor_scalar(y0i[:], sh[:], -1, 0x5F3759DF, ALU.mult,
                                    ALU.add)
            y = y0i[:].bitcast(f32)
            for it in range(2):
                y2 = pool_sm.tile([N, 1], f32, tag="ln_y2")
                nc.vector.tensor_tensor(y2[:], y, y, ALU.mult)
                t = pool_sm.tile([N, 1], f32, tag="ln_t")
                nc.vector.tensor_tensor(t[:], var_eps, y2[:], ALU.mult)
                fct = pool_sm.tile([N, 1], f32, tag="ln_f")
                nc.vector.tensor_scalar(fct[:], t[:], -0.5, 1.5, ALU.mult,
                                        ALU.add)
                yn = pool_sm.tile([N, 1], f32, tag="ln_yn")
                nc.vector.tensor_tensor(yn[:], y, fct[:], ALU.mult)
                y = yn[:]
            return y

        # --- constants -------------------------------------------------
        def cload(name, d, shape, dt=bf16):
            t = pool_c.tile(list(shape), dt, tag=name)
            nc.sync.dma_start(t[:], d[:])
            return t

        crow_t = cload("crow", crow_d, (1, 1984))
        cmat_t = cload("cmat", cmat_d, (N, 1472))
        nemb = cload("nemb", nemb_d, (N, H), f32)
        onesr = crow_t[:, 0:128]
        ones512 = crow_t[:, 128:640]
        wrow = crow_t[:, 640:768]
        ww = crow_t[:, 768:1280]
        bmerge = crow_t[:, 1280:1792]
        eb1r = crow_t[:, 1792:1856]
        eb2d = crow_t[:, 1856:1984]
        tsr = cmat_t[:, 0:128]
        nmaskb_t = pool_c.tile([N, N], bf16, tag="nmk")
        nc.vector.tensor_copy(nmaskb_t[:], cmat_t[:, 128:256])
        nmaskb = nmaskb_t[:]
        identb_t = pool_c.tile([N, N], bf16, tag="idb")
        nc.vector.tensor_copy(identb_t[:], cmat_t[:, 256:384])
        identb = identb_t[:]
        ident64 = identb_t[0:EHD, 0:EHD]
        selb = cmat_t[:, 384:448]
        bident = cmat_t[:, 448:960]
        wtopo = cmat_t[:, 960:1472]

        # --- x0 --------------------------------------------------------
        xp = pool_pb.tile([N, H], f32, tag="pb")
        nc.tensor.matmul(xp[:], tsr, wtopo, start=True, stop=False)
        nc.tensor.matmul(xp[:], wrow, ww, start=False, stop=False)
        nc.tensor.matmul(xp[:], onesr, bmerge, start=False, stop=True)
        x = pool_x.tile([N, H], f32, tag="x")
        nc.vector.tensor_tensor(x[:], xp[:], nemb[:], ALU.add)

        def layernorm_to_hT(x_ap, htag, st12=None):
            """Return hT [128, FC, 128] bf16 = LayerNorm(x)^T:
            xb = (x - mean)*rstd in one dual-pointer op, then plain PE
            transposes (ACT engine untouched)."""
            if st12 is None:
                st12 = pool_sm.tile([N, 6], f32, tag="ln_st6")
                nc.vector.bn_stats(st12[:], x_ap)
            st2 = pool_sm.tile([N, 2], f32, tag="ln_st2")
            nc.vector.bn_aggr(st2[:], st12[:])
            nm = pool_sm.tile([N, 1], f32, tag="ln_nm")
            nc.vector.tensor_scalar(nm[:], st2[:, 0:1], -1.0, None, ALU.mult)
            ve = pool_sm.tile([N, 1], f32, tag="ln_ve")
            nc.vector.tensor_scalar(ve[:], st2[:, 1:2], 1e-6, None, ALU.add)
            rstd = ln_newton(ve[:])
            xb = pool_t.tile([N, H], bf16, tag="xb_" + htag)
            nc.vector.tensor_scalar(xb[:], x_ap, nm[:], rstd, ALU.add,
                                    ALU.mult)
            hT = pool_a.tile([128, FC, 128], bf16, tag=htag)
            for c in range(FC):
                tp = pool_ps.tile([128, 128], bf16, tag="ps")
                nc.tensor.transpose(tp[:], xb[:, c * 128:(c + 1) * 128],
                                    identb)
                nc.vector.tensor_copy(hT[:, c, :], tp[:])
            return hT

        # ================== trunk layers ==============================
        next_st12 = None
        for d in range(TRUNK_LAYERS):
            wq_t = pool_w.tile([128, FC, H], bf16, tag="wq")
            wk_t = pool_w.tile([128, FC, H], bf16, tag="wk")
            wv_t = pool_w.tile([128, FC, H], bf16, tag="wv")
            wo_t = pool_w.tile([128, FC, H], bf16, tag="wo")
            for wi, wt in enumerate((wq_t, wk_t, wv_t, wo_t)):
                nc.sync.dma_start(
                    wt[:],
                    wqkvo_d[:].rearrange("d w (c p) n -> d w p c n",
                                         p=128)[d, wi])
            w1_t = pool_w.tile([128, FC, MH], bf16, tag="w1")
            w2_t = pool_w.tile([128, MC, H], bf16, tag="w2")
            w1v = w1_d[:].rearrange("d (c p) m -> d p c m", p=128)
            w2v = w2_d[:].rearrange("d (c p) n -> d p c n", p=128)
            nc.sync.dma_start(w1_t[:, :, 0:MH // 2], w1v[d, :, :, 0:MH // 2])
            nc.sync.dma_start(w2_t[:, 0:MC // 2, :], w2v[d, :, 0:MC // 2, :])
            nc.sync.dma_start(w1_t[:, :, MH // 2:MH],
                              w1v[d, :, :, MH // 2:MH])
            nc.sync.dma_start(w2_t[:, MC // 2:MC, :], w2v[d, :, MC // 2:MC, :])
            bias_t = pool_w.tile([1, 4608], bf16, tag="biases")
            for off in (0, 1024, 2048, 3072, 4096):
                end = min(off + 1024, 4608)
                nc.sync.dma_start(bias_t[:, off:end], biases_d[d:d + 1, off:end])

            # ---- LN1 -> hT -------------------------------------------
            hT = layernorm_to_hT(x[:], "hT", st12=next_st12)

            # ---- Q, K directly transposed (+bias, pre-scaled) --------
            def qkT(wi, boff, out_tag, drain_act):
                qT = pool_a.tile([128, FC, 128], bf16, tag=out_tag)
                for o in range(FC):
                    pq = pool_ps.tile([128, 128], f32, tag="ps")
                    for c in range(FC):
                        nc.tensor.matmul(
                            pq[:], (wq_t if wi == 0 else wk_t)[:, c, o * 128:(o + 1) * 128],
                            hT[:, c, :], start=(c == 0), stop=False)
                    nc.tensor.matmul(
                        pq[:], bias_t[:, boff + o * 128:boff + (o + 1) * 128],
                        onesr, start=False, stop=True)
                    if drain_act:
                        nc.scalar.copy(qT[:, o, :], pq[:])
                    else:
                        nc.vector.tensor_copy(qT[:, o, :], pq[:])
                return qT

            QT = qkT(0, 0, "QT", False)
            KT = qkT(1, 512, "KT", True)

            # ---- V natural -------------------------------------------
            vp = pool_pb.tile([N, H], f32, tag="pb")
            for c in range(FC):
                nc.tensor.matmul(vp[:], hT[:, c, :], wv_t[:, c, :],
                                 start=(c == 0), stop=False)
            nc.tensor.matmul(vp[:], onesr, bias_t[:, 1024:1536],
                             start=False, stop=True)
            V = pool_a.tile([N, H], bf16, tag="V")
            nc.vector.tensor_copy(V[:], vp[:])

            # ---- attention: 8 heads, 2 halves, software-pipelined ----
            aggT = pool_a.tile([128, FC, 128], bf16, tag="aggT")
            atp = pool_ps.tile([128, FC, 128], f32, tag="ps")
            sps = []
            for half in range(2):
                sp = pool_ps.tile([128, 2, 2, 128], f32, tag="ps")
                sps.append(sp)
                for ci in range(2):
                    c = 2 * half + ci
                    for hh in range(2):
                        po = hh * 64
                        nc.tensor.matmul(sp[:, ci, hh, :],
                                         QT[po:po + 64, c, :],
                                         KT[po:po + 64, c, :],
                                         start=(ci == 0 and hh == 0),
                                         stop=False)
                        nc.tensor.matmul(sp[:, ci, hh, :], identb, nmaskb,
                                         start=False,
                                         stop=(ci == 1 and hh == 1))
            for half in range(2):
                sp = sps[half]
                P = pool_t.tile([128, 2, 2, 128], bf16, tag=f"P{half}")
                nc.scalar.activation(P[:], sp[:], AF.Exp)
                zs = pool_sm.tile([128, 4], f32, tag=f"zs{half}")
                rec = pool_sm.tile([128, 4], f32, tag=f"rec{half}")
                nc.vector.reduce_sum(zs[:, 0:2], P[:, 0, :, :], axis=AX.X)
                nc.vector.reciprocal(rec[:, 0:2], zs[:, 0:2])
                nc.vector.reduce_sum(zs[:, 2:4], P[:, 1, :, :], axis=AX.X)
                nc.vector.reciprocal(rec[:, 2:4], zs[:, 2:4])
                Pn = pool_t.tile([128, 2, 2, 128], bf16, tag=f"Pn{half}")
                ptp = pool_ps.tile([128, 2, 2, 128], bf16, tag="ps")
                for ci in range(2):
                    for hh in range(2):
                        k = 2 * ci + hh
                        nc.vector.tensor_scalar(Pn[:, ci, hh, :],
                                                P[:, ci, hh, :],
                                                rec[:, k:k + 1], None,
                                                ALU.mult)
                        nc.tensor.transpose(ptp[:, ci, hh, :],
                                            Pn[:, ci, hh, :], identb)
                PT = pool_t.tile([128, 2, 2, 128], bf16, tag=f"PT{half}")
                nc.vector.tensor_copy(PT[:], ptp[:])
                for ci in range(2):
                    c = 2 * half + ci
                    for hh in range(2):
                        po = hh * 64
                        head = 2 * c + hh
                        nc.tensor.matmul(atp[po:po + 64, c, :],
                                         V[:, head * 64:head * 64 + 64],
                                         PT[:, ci, hh, :],
                                         start=True, stop=True)
                    if ci == 0:
                        nc.vector.tensor_copy(aggT[:, c, :],
                                              atp[:, c, :])
                    else:
                        nc.scalar.copy(aggT[:, c, :], atp[:, c, :])

            # ---- O proj + residual -----------------------------------
            op = pool_pb.tile([N, H], f32, tag="pb")
            for c in range(FC):
                nc.tensor.matmul(op[:], aggT[:, c, :], wo_t[:, c, :],
                                 start=(c == 0), stop=False)
            nc.tensor.matmul(op[:], onesr, bias_t[:, 1536:2048],
                             start=False, stop=True)
            x1 = pool_x.tile([N, H], f32, tag="x")
            nc.vector.tensor_tensor(x1[:], op[:], x[:], ALU.add)
            x = x1

            # ---- LN2 + FFN -------------------------------------------
            h2T = layernorm_to_hT(x[:], "h2T")

            mid = pool_a.tile([N, MH], bf16, tag="mid")
            for q in range(4):
                off = q * 512
                mp = pool_pb.tile([N, 512], f32, tag="pb")
                for c in range(FC):
                    nc.tensor.matmul(mp[:], h2T[:, c, :],
                                     w1_t[:, c, off:off + 512],
                                     start=(c == 0), stop=False)
                nc.tensor.matmul(mp[:], onesr,
                                 bias_t[:, 2048 + off:2048 + off + 512],
                                 start=False, stop=True)
                nc.scalar.activation(mid[:, off:off + 512], mp[:],
                                     AF.Gelu_apprx_tanh)

            midT = pool_a.tile([128, MC, 128], bf16, tag="midT")
            for t in range(MC):
                tp = pool_ps.tile([128, 128], bf16, tag="ps")
                nc.tensor.transpose(tp[:], mid[:, t * 128:(t + 1) * 128],
                                    identb)
                nc.vector.tensor_copy(midT[:, t, :], tp[:])

            fp = pool_pb.tile([N, H], f32, tag="pb")
            for t in range(MC):
                nc.tensor.matmul(fp[:], midT[:, t, :], w2_t[:, t, :],
                                 start=(t == 0), stop=False)
            nc.tensor.matmul(fp[:], onesr, bias_t[:, 4096:4608],
                             start=False, stop=True)
            x2 = pool_x.tile([N, H], f32, tag="x")
            nc.vector.tensor_tensor(x2[:], fp[:], x[:], ALU.add)
            x = x2

        if not RUN_CLASSIFIER:
            nc.sync.dma_start(lout_d[:], x[0:16, 0:512])
        else:
            # ================== edge classifier =======================
            ew1_view = ew1_d[:].rearrange("(s c p) n -> s p c n", s=3, p=128)
            Aw = pool_c.tile([128, FC, EHD], bf16, tag="Aw")
            nc.sync.dma_start(Aw[:], ew1_view[0])
            Bw = pool_c.tile([128, FC, EHD], bf16, tag="Bw")
            nc.sync.dma_start(Bw[:], ew1_view[1])
            Cw = pool_c.tile([128, FC, EHD], bf16, tag="Cw")
            nc.sync.dma_start(Cw[:], ew1_view[2])
            ew2bd = pool_c.tile([2 * EHD, 2 * EHD], bf16, tag="ew2bd")
            nc.sync.dma_start(ew2bd[:], ew2bd_d[:])
            ew3dd = pool_c.tile([2 * EHD, 2], bf16, tag="ew3dd")
            nc.sync.dma_start(ew3dd[:], ew3dd_d[:])
            bones = pool_c.tile([4, 512], bf16, tag="bones")
            nc.sync.dma_start(bones[:], blockones_d[:])

            # x in bf16 + transposed
            xb = pool_a.tile([N, H], bf16, tag="xb")
            nc.vector.tensor_copy(xb[:], x[:])
            xT = pool_a.tile([128, FC, 128], bf16, tag="xT")
            for c in range(FC):
                tp = pool_ps.tile([128, 128], bf16, tag="ps")
                nc.tensor.transpose(tp[:], xb[:, c * 128:(c + 1) * 128],
                                    identb)
                nc.vector.tensor_copy(xT[:, c, :], tp[:])
            # selected columns (this core's 64 rows), transposed layout.
            # fp32 copy feeds tensor_scalar ptrs; bf16 copy feeds matmuls.
            xselF = pool_a.tile([128, FC, ROWS], f32, tag="xselF")
            xselT = pool_a.tile([128, FC, ROWS], bf16, tag="xselT")
            for c in range(FC):
                sq = pool_ps.tile([128, ROWS], f32, tag="ps")
                nc.tensor.matmul(sq[:], xb[:, c * 128:(c + 1) * 128], selb,
                                 start=True, stop=True)
                nc.vector.tensor_copy(xselF[:, c, :], sq[:])
                nc.vector.tensor_copy(xselT[:, c, :], xselF[:, c, :])

            # u = A^T xsel  [EHD, ROWS] -> transpose -> uT_nat [ROWS, EHD]
            up = pool_ps.tile([EHD, ROWS], f32, tag="ps")
            for c in range(FC):
                nc.tensor.matmul(up[:], Aw[:, c, :], xselT[:, c, :],
                                 start=(c == 0), stop=(c == FC - 1))
            u_sb = pool_a.tile([EHD, ROWS], bf16, tag="u_sb")
            nc.vector.tensor_copy(u_sb[:], up[:])
            utp_all = pool_ps.tile([4, 16, EHD], bf16, tag="ps")
            for g2i in range(16):
                nc.tensor.transpose(utp_all[:, g2i, :],
                                    u_sb[:, 4 * g2i:4 * g2i + 4], ident64)
            uT_all = pool_a.tile([4, 16, EHD], bf16, tag="uT_all")
            nc.vector.tensor_copy(uT_all[:], utp_all[:])
            if K_DBG:
                uc = pool_t.tile([EHD, ROWS], f32, tag="uc")
                nc.vector.tensor_copy(uc[:], u_sb[:])
                nc.sync.dma_start(dbg["dbg_u"][:], uc[:])

            # v = B^T x (+eb1)  [EHD, N] -> transpose -> vT2 [N, 2*EHD] (dup)
            vp2 = pool_ps.tile([EHD, N], f32, tag="ps")
            for c in range(FC):
                nc.tensor.matmul(vp2[:], Bw[:, c, :], xT[:, c, :],
                                 start=(c == 0), stop=False)
            nc.tensor.matmul(vp2[:], eb1r, onesr, start=False, stop=True)
            vn = pool_a.tile([EHD, N], bf16, tag="vn")
            nc.vector.tensor_copy(vn[:], vp2[:])
            vtp = pool_ps.tile([N, 128], bf16, tag="ps")
            nc.tensor.transpose(vtp[:, 0:EHD], vn[:], ident64)
            vT2 = pool_a.tile([N, 2 * EHD], bf16, tag="vT2")
            nc.vector.tensor_copy(vT2[:, 0:EHD], vtp[:, 0:EHD])
            nc.vector.tensor_copy(vT2[:, EHD:2 * EHD], vtp[:, 0:EHD])
            if K_DBG:
                vc = pool_t.tile([N, 2 * EHD], f32, tag="vc")
                nc.vector.tensor_copy(vc[:], vT2[:])
                nc.sync.dma_start(dbg["dbg_v"][:], vc[:])
                xsc = pool_t.tile([128, FC, ROWS], f32, tag="xsc")
                nc.vector.tensor_copy(xsc[:], xselF[:])
                nc.sync.dma_start(dbg["dbg_xsel"][:],
                                  xsc[:].rearrange("p a b -> p (a b)"))
                xtc = pool_t.tile([128, FC, 128], f32, tag="xtc")
                nc.vector.tensor_copy(xtc[:], xT[:])
                nc.sync.dma_start(dbg["dbg_xT"][:],
                                  xtc[:].rearrange("p a b -> p (a b)"))

            # two row-groups per pass: partitions 0-63 carry group A's EHD
            # lanes, 64-127 group B's.
            for sg in range(NG // 2):
                s0 = 8 * sg
                zv = pool_pb.tile([128, 4, 128], f32, tag="pb")
                cs_all = []
                for c in range(FC):
                    cs = pool_t.tile([128, 4, 128], bf16, tag=f"cs{c}")
                    cs_all.append(cs)
                    for i in range(4):
                        for hf in range(2):
                            eng = nc.gpsimd if (
                                i == 3 or (i == 2 and hf == 1 and c % 2 == 0)
                            ) else nc.vector
                            eng.tensor_scalar(
                                cs[:, i, hf * EHD:(hf + 1) * EHD],
                                Cw[:, c, :],
                                xselF[:, c, s0 + 4 * hf + i:
                                      s0 + 4 * hf + i + 1],
                                None, ALU.mult)
                for i in range(4):
                    for c in range(FC):
                        nc.tensor.matmul(zv[:, i, :], cs_all[c][:, i, :],
                                         xT[:, c, :],
                                         start=(i == 0 and c == 0),
                                         stop=False)
                zvf = zv[:].rearrange("p a b -> p (a b)")
                for hf in range(2):
                    nc.tensor.matmul(zvf[hf * EHD:(hf + 1) * EHD, :],
                                     uT_all[:, 2 * sg + hf, :],
                                     bones[:], start=False, stop=False)
                nc.tensor.matmul(zvf, vT2[:], bident, start=False,
                                 stop=True)
                if K_DBG and sg == 0:
                    zvc = pool_t.tile([128, 512], f32, tag="zvc")
                    nc.vector.tensor_copy(zvc[:], zvf)
                    nc.sync.dma_start(dbg["dbg_zv"][:], zvc[:])
                g1 = pool_t.tile([128, 512], bf16, tag="g1")
                nc.scalar.activation(g1[:], zvf, AF.Gelu_apprx_tanh)
                if K_DBG and sg == 0:
                    g1c = pool_t.tile([128, 512], f32, tag="g1c")
                    nc.vector.tensor_copy(g1c[:], g1[:])
                    nc.sync.dma_start(dbg["dbg_g1"][:], g1c[:])
                g2p = pool_pb.tile([128, 512], f32, tag="pb")
                nc.tensor.matmul(g2p[:], ew2bd[:], g1[:], start=True,
                                 stop=False)
                nc.tensor.matmul(g2p[:], eb2d, ones512, start=False,
                                 stop=True)
                g2 = pool_t.tile([128, 512], bf16, tag="g2")
                nc.scalar.activation(g2[:], g2p[:], AF.Gelu_apprx_tanh)
                if K_DBG and sg == 0:
                    g2c = pool_t.tile([128, 512], f32, tag="g2c")
                    nc.vector.tensor_copy(g2c[:], g2[:])
                    nc.sync.dma_start(dbg["dbg_g2"][:], g2c[:])
                lp = pool_ps.tile([2, 512], f32, tag="ps")
                nc.tensor.matmul(lp[:], ew3dd[:], g2[:], start=True,
                                 stop=True)
                lr2 = pool_o.tile([2, 512], f32, tag="lr2")
                nc.scalar.copy(lr2[:], lp[:])
                nc.sync.dma_start(lout_d[2 * sg:2 * sg + 2, :], lr2[:])

    nc.compile()
    return nc


_CACHE = {}


def _get_nc():
    if "nc" not in _CACHE:
        _CACHE["nc"] = build_program()
    return _CACHE["nc"]


def _prep_in_maps(inputs):
    f = lambda k: np.asarray(inputs[k], dtype=np.float32)
    bf = lambda a: np.ascontiguousarray(np.asarray(a, dtype=np.float32)
                                        .astype(BF))
    topo = f("topo")
    weight = f("weight")
    tsym = topo + topo.transpose(0, 2, 1)
    ident = np.eye(N, dtype=np.float32)

    # crow blob [1, 1984]: onesrow | ones512 | wrow(per-core) | ww | bmerge
    #                      | eb1 | eb2dup
    crow = np.zeros((1, 1984), np.float32)
    crow[0, 0:128] = 1.0
    crow[0, 128:640] = 1.0
    crow[0, 768:1280] = f("w_w")[0]
    crow[0, 1280:1792] = f("b_topo") + f("b_w")
    crow[0, 1792:1856] = f("eb1")
    crow[0, 1856:1984] = np.tile(f("eb2"), 2)

    # cmat blob [128, 1472]: tsr | nmask | ident | sel(per-core)
    #                        | blockident | wtopo
    cmat = np.zeros((N, 1472), np.float32)
    cmat[:, 256:384] = ident
    cmat[:, 448:960] = np.tile(ident, (1, 4))
    cmat[:, 960:1472] = f("w_topo")

    bones = np.zeros((4, 512), np.float32)
    for i in range(4):
        bones[i, i * 128:(i + 1) * 128] = 1.0
    sels = []
    for hh in range(2):
        sl = np.zeros((N, ROWS), np.float32)
        sl[hh * ROWS + np.arange(ROWS), np.arange(ROWS)] = 1.0
        sels.append(sl)

    wqkvo = np.stack([f("wq") * SCALE, f("wk"), f("wv"), f("wo")], axis=1)
    biases = np.concatenate([
        f("bq") * SCALE, f("bk"), f("bv"), f("bo"), f("b1"), f("b2")], axis=1)

    shared = dict(
        blockones=bf(bones),
        nemb=np.ascontiguousarray(f("n_emb")),
        wqkvo=bf(wqkvo), biases=bf(biases),
        w1=bf(f("w1")), w2=bf(f("w2")),
        ew1=bf(f("ew1")),
        ew2bd=bf(np.block([
            [f("ew2"), np.zeros((EHD, EHD), np.float32)],
            [np.zeros((EHD, EHD), np.float32), f("ew2")]])),
        ew3dd=bf(np.block([
            [f("ew3").reshape(EHD, 1), np.zeros((EHD, 1), np.float32)],
            [np.zeros((EHD, 1), np.float32), f("ew3").reshape(EHD, 1)]])),
    )
    in_maps = []
    for core in range(NCORES):
        g, hh = core // 2, core % 2
        m = dict(shared)
        cr = crow.copy()
        cr[0, 640:768] = weight[g]
        cm = cmat.copy()
        cm[:, 0:128] = tsym[g]
        cm[:, 128:256] = np.where(tsym[g] > 0, 0.0, NEGM)
        cm[:, 384:448] = sels[hh]
        m["crow"] = bf(cr)
        m["cmat"] = bf(cm)
        in_maps.append(m)
    return in_maps, tsym


def _postprocess(results, tsym, eb3):
    p = np.zeros((B, N, N), dtype=np.float32)
    for core in range(NCORES):
        g, hh = core // 2, core % 2
        logits = results[core]["lout"].reshape(16, 4, 128) \
            .reshape(ROWS, N).astype(np.float32) + eb3
        p[g, hh * ROWS:(hh + 1) * ROWS, :] = 1.0 / (1.0 + np.exp(-logits))
    p = 0.5 * (p + p.transpose(0, 2, 1))
    p *= (1.0 - np.eye(N, dtype=np.float32))
    p *= (tsym > 0).astype(np.float32)
    return p


def run(inputs, **spmd_kwargs):
    nc = _get_nc()
    in_maps, tsym = _prep_in_maps(inputs)
    eb3 = float(np.asarray(inputs["eb3"], dtype=np.float32).reshape(-1)[0])
    res = run_bass_kernel_spmd(nc, in_maps, list(range(NCORES)), **spmd_kwargs)
    return _postprocess(res.results, tsym, eb3), res


def kernel(**inputs):
    out, _ = run(inputs)
    return out
